# revision 25
# baseline (speedup 1.0000x reference)
"""AgriMatcher Trainium2 kernel: point-matching network + weighted-DLT homography.

Strategy: pure data-parallel over batch B=64 across 8 NeuronCores (8 images
per core). The device computes the heavy network (feature-compression MLP,
PointNet encoder, weighting head) and accumulates, per image, the 9x9
weighted Gram matrix M = sum_n w_n q_n q_n^T with
q = [sx, sy, 1, dx, dy, dx*sx, dx*sy, dy*sx, dy*sy] over Hartley-normalized
points. The host assembles AtWA/AtWb from M, solves the 8x8 system, and
composes the final 3x3 homographies (O(B * 8^3) flops, negligible).

Numerics: matmuls in bf16 with fp32 PSUM accumulation; DLT Gram in fp32;
LayerNorm folded into centered fc1 weights (mean-free), rstd via
Newton-iterated fast inverse sqrt on the Vector engine; sigmoid computed as
0.5*tanh(0.5x)+0.5 so every ScalarE function used (gelu/tanh/relu/copy/abs)
lives in one activation table set (no table-switch stalls).
"""

import numpy as np
import ml_dtypes

import concourse.bass as bass
import concourse.mybir as mybir
import concourse.tile as tile
from concourse import bacc, bass_utils
from concourse.masks import make_identity

F32 = mybir.dt.float32
BF16 = mybir.dt.bfloat16
I32 = mybir.dt.int32
AF = mybir.ActivationFunctionType
OP = mybir.AluOpType
AX = mybir.AxisListType

B, N, C = 64, 4096, 128
HID, COMP = 128, 32
NCORES = 8
BL = B // NCORES          # images per core
TILE = 1024               # points per tile
NT = N // TILE            # tiles per image (4)
NCH = TILE // 128         # 128-pt chunks per tile (8)
NTC = BL * NT             # tiles per core (32)
NC32 = N // 128           # 128-pt chunks per image (32)
EPS = 1e-5
REG = 1e-4
MAGIC = 0x5F3759DF

BF = ml_dtypes.bfloat16


def build():
    nc = bacc.Bacc("TRN2", target_bir_lowering=False, debug=False,
                   num_devices=NCORES)

    featA = nc.dram_tensor("featA", [BL, C, N], BF16, kind="ExternalInput").ap()
    featB = nc.dram_tensor("featB", [BL, C, N], BF16, kind="ExternalInput").ap()
    posT = nc.dram_tensor("posT", [BL, 4, N], BF16, kind="ExternalInput").ap()
    posn = nc.dram_tensor("posn", [128, BL, NC32, 4], F32,
                          kind="ExternalInput").ap()
    # packed params (see host prep below for layouts)
    w1dT = nc.dram_tensor("w1dT", [128, 64], BF16, kind="ExternalInput").ap()
    w1mT = nc.dram_tensor("w1mT", [128, 64], BF16, kind="ExternalInput").ap()
    b1c = nc.dram_tensor("b1c", [64, 1], F32, kind="ExternalInput").ap()
    g_col = nc.dram_tensor("g_col", [64, 1], F32, kind="ExternalInput").ap()
    b_ln = nc.dram_tensor("b_ln", [64, 1], F32, kind="ExternalInput").ap()
    wenc0 = nc.dram_tensor("wenc0", [68, 128], BF16, kind="ExternalInput").ap()
    benc0 = nc.dram_tensor("benc0", [128, 1], F32, kind="ExternalInput").ap()
    wenc1 = nc.dram_tensor("wenc1", [128, 128], BF16, kind="ExternalInput").ap()
    benc1 = nc.dram_tensor("benc1", [128, 1], F32, kind="ExternalInput").ap()
    wenc2 = nc.dram_tensor("wenc2", [128, 128], BF16, kind="ExternalInput").ap()
    benc2 = nc.dram_tensor("benc2", [128, 1], F32, kind="ExternalInput").ap()
    w0a = nc.dram_tensor("w0a", [128, 128], BF16, kind="ExternalInput").ap()
    w0b = nc.dram_tensor("w0b", [128, 128], BF16, kind="ExternalInput").ap()
    bh0 = nc.dram_tensor("bh0", [128, 1], F32, kind="ExternalInput").ap()
    wh1 = nc.dram_tensor("wh1", [128, 64], BF16, kind="ExternalInput").ap()
    bh1 = nc.dram_tensor("bh1", [64, 1], F32, kind="ExternalInput").ap()
    w2col = nc.dram_tensor("w2col", [64, 1], BF16, kind="ExternalInput").ap()
    tb2 = nc.dram_tensor("tb2", [128, 1], F32, kind="ExternalInput").ap()

    out = nc.dram_tensor("out", [BL, 9, 9], F32, kind="ExternalOutput").ap()

    with tile.TileContext(nc) as tc:
        with (
            tc.tile_pool(name="const", bufs=1) as cp,
            tc.tile_pool(name="persist", bufs=1) as pp,
            tc.tile_pool(name="work", bufs=3) as wp,
            tc.tile_pool(name="feat", bufs=4) as fp,
            tc.tile_pool(name="ps", bufs=2, space="PSUM") as ps,
            tc.tile_pool(name="psb", bufs=2, space="PSUM") as psb,
        ):
            # ---- constants ----
            ident = cp.tile([128, 128], BF16)
            make_identity(nc, ident)

            def cload(ap_in, shape, dtype):
                t = cp.tile(shape, dtype, tag=ap_in.tensor.name)
                nc.sync.dma_start(out=t, in_=ap_in)
                return t

            w1dT_t = cload(w1dT, [128, 64], BF16)
            w1mT_t = cload(w1mT, [128, 64], BF16)
            b1c_t = cload(b1c, [64, 1], F32)
            g_col_t = cload(g_col, [64, 1], F32)
            b_ln_t = cload(b_ln, [64, 1], F32)
            wenc0_t = cload(wenc0, [68, 128], BF16)
            benc0_t = cload(benc0, [128, 1], F32)
            wenc1_t = cload(wenc1, [128, 128], BF16)
            benc1_t = cload(benc1, [128, 1], F32)
            wenc2_t = cload(wenc2, [128, 128], BF16)
            benc2_t = cload(benc2, [128, 1], F32)
            w0a_t = cload(w0a, [128, 128], BF16)
            w0b_t = cload(w0b, [128, 128], BF16)
            bh0_t = cload(bh0, [128, 1], F32)
            wh1_t = cload(wh1, [128, 64], BF16)
            bh1_t = cload(bh1, [64, 1], F32)
            w2col_t = cload(w2col, [64, 1], BF16)
            tb2_t = cload(tb2, [128, 1], F32)

            posn_sb = pp.tile([128, BL, NC32, 4], F32)
            nc.sync.dma_start(out=posn_sb, in_=posn)

            # ---- persistent state ----
            hc_all = pp.tile([128, NTC, NCH, 64], BF16)   # centered fc1 out (PM)
            s2_all = pp.tile([128, NTC, NCH, 1], F32)     # sum(hc^2) per point
            rstd_all = pp.tile([128, NTC * NCH], F32)
            w_all = pp.tile([128, BL, NC32], F32)         # per-point weights
            gparts = pp.tile([128, BL, NT], BF16)         # per-tile max partials

            def p1_load(img, ti):
                p0 = ti * TILE
                faT = fp.tile([128, TILE], BF16, tag="faT")
                fbT = fp.tile([128, TILE], BF16, tag="fbT")
                nc.sync.dma_start(out=faT, in_=featA[img, :, p0:p0 + TILE])
                nc.sync.dma_start(out=fbT, in_=featB[img, :, p0:p0 + TILE])
                return faT, fbT

            def p1_s0(st):
                faT, fbT = st["f"]
                d_t = wp.tile([128, TILE], BF16, tag="d")
                nc.vector.tensor_sub(d_t, faT, fbT)
                st["d"] = d_t

            def p1_s1(st):
                d_t = st["d"]
                nc.scalar.activation(d_t, d_t, AF.Abs)

            def p1_s2(st):
                faT, fbT = st["f"]
                m_t = wp.tile([128, TILE], BF16, tag="m")
                nc.vector.tensor_mul(m_t, faT, fbT)
                st["m"] = m_t

            def p1_s3(st):
                d_t, m_t = st["d"], st["m"]
                h_ps = ps.tile([64, TILE], F32, tag="big")
                for half in range(2):
                    sl = slice(half * 512, half * 512 + 512)
                    nc.tensor.matmul(h_ps[:, sl], w1dT_t, d_t[:, sl],
                                     start=True, stop=False)
                    nc.tensor.matmul(h_ps[:, sl], w1mT_t, m_t[:, sl],
                                     start=False, stop=True)
                st["h_ps"] = h_ps

            def p1_s4(st):
                h_sb = wp.tile([64, TILE], BF16, tag="h_sb")
                nc.scalar.activation(h_sb, st["h_ps"], AF.Identity,
                                     bias=b1c_t)
                st["h_sb"] = h_sb

            def p1_s5(st):
                hp_ps = psb.tile([128, NCH, 64], BF16, tag="tp")
                h_sb = st["h_sb"]
                for j in range(NCH):
                    nc.tensor.transpose(hp_ps[:, j, :],
                                        h_sb[:, j * 128:(j + 1) * 128],
                                        ident[:64, :64])
                st["hp_ps"] = hp_ps

            def p1_s6(st):
                t = st["t"]
                hcv = hc_all[:, t].rearrange("p a b -> p (a b)")
                nc.vector.tensor_copy(
                    hcv, st["hp_ps"].rearrange("p a b -> p (a b)"))
                sq = wp.tile([128, NCH * 64], BF16, tag="sq")
                nc.vector.tensor_mul(sq, hcv, hcv)
                nc.vector.reduce_sum(out=s2_all[:, t],
                                     in_=sq.rearrange("p (a b) -> p a b",
                                                      a=NCH), axis=AX.X)

            P1_STAGES = [p1_s0, p1_s1, p1_s2, p1_s3, p1_s4, p1_s5, p1_s6]

            def newton_all():
                # rstd = (s2/64 + eps)^-1/2 via fast-invsqrt + 3 Newton steps
                s2f = s2_all.rearrange("p a b c -> p (a b c)")
                vp = vp_all
                yv = rstd_all
                u_t = u_all
                nc.vector.tensor_scalar(vp, s2f, 1.0 / 64.0, EPS,
                                        op0=OP.mult, op1=OP.add)
                nc.vector.tensor_scalar(yv.bitcast(I32), vp.bitcast(I32), 1,
                                        None, op0=OP.arith_shift_right)
                nc.vector.tensor_scalar(yv.bitcast(I32), yv.bitcast(I32),
                                        0xFFFFFFFF, None, op0=OP.bitwise_xor)
                nc.vector.tensor_scalar(yv.bitcast(I32), yv.bitcast(I32),
                                        MAGIC + 1, None, op0=OP.add)
                for _ in range(3):
                    nc.vector.tensor_mul(u_t, yv, yv)
                    nc.vector.tensor_mul(u_t, u_t, vp)
                    nc.vector.tensor_scalar(u_t, u_t, -0.5, 1.5,
                                            op0=OP.mult, op1=OP.add)
                    nc.vector.tensor_mul(yv, yv, u_t)

            def e_s0(st):
                t = st["t"]
                diag_t = wp.tile([128, NCH, 128], BF16, tag="diag")
                ident_b = bass.AP(tensor=ident.tensor, offset=ident.offset,
                                  ap=[ident.ap[0], [0, NCH], ident.ap[1]])
                rstd_b = rstd_all[:, t * NCH:(t + 1) * NCH]
                rstd_bc = bass.AP(tensor=rstd_b.tensor, offset=rstd_b.offset,
                                  ap=[rstd_b.ap[0], rstd_b.ap[1], [0, 128]])
                nc.vector.tensor_tensor(out=diag_t, in0=ident_b, in1=rstd_bc,
                                        op=OP.mult)
                st["diag"] = diag_t

            def e_s1(st):
                t = st["t"]
                ycm_ps = psb.tile([64, TILE], BF16, tag="tp")
                diag_t = st["diag"]
                for j in range(NCH):
                    nc.tensor.transpose(ycm_ps[:, j * 128:(j + 1) * 128],
                                        hc_all[:, t, j, :], diag_t[:, j, :])
                st["ycm"] = ycm_ps

            def e_s2(st):
                img, ti = st["img"], st["ti"]
                p0 = ti * TILE
                hg = wp.tile([68, TILE], BF16, tag="hg")
                nc.sync.dma_start(out=hg[64:68, :],
                                  in_=posT[img, :, p0:p0 + TILE])
                nc.scalar.activation(hg[0:64, :], st["ycm"], AF.Gelu,
                                     bias=b_ln_t, scale=g_col_t)
                st["hg"] = hg

            def e_s3(st):
                e0 = ps.tile([128, TILE], F32, tag="big")
                hg = st["hg"]
                for half in range(2):
                    sl = slice(half * 512, half * 512 + 512)
                    nc.tensor.matmul(e0[:, sl], wenc0_t, hg[:, sl],
                                     start=True, stop=True)
                st["e0"] = e0

            def e_s4(st):
                x1 = wp.tile([128, TILE], BF16, tag="x1")
                nc.scalar.activation(x1, st["e0"], AF.Relu, bias=benc0_t)
                st["x1"] = x1

            def e_s5(st):
                e1 = ps.tile([128, TILE], F32, tag="big")
                x1 = st["x1"]
                for half in range(2):
                    sl = slice(half * 512, half * 512 + 512)
                    nc.tensor.matmul(e1[:, sl], wenc1_t, x1[:, sl],
                                     start=True, stop=True)
                st["e1"] = e1

            def e_s6(st):
                x2 = wp.tile([128, TILE], BF16, tag="x2")
                nc.vector.tensor_scalar(x2, st["e1"], benc1_t, 0.0,
                                        op0=OP.add, op1=OP.max)
                st["x2"] = x2

            def e_s7(st):
                e2 = ps.tile([128, TILE], F32, tag="big")
                x2 = st["x2"]
                for half in range(2):
                    sl = slice(half * 512, half * 512 + 512)
                    nc.tensor.matmul(e2[:, sl], wenc2_t, x2[:, sl],
                                     start=True, stop=True)
                st["e2"] = e2

            def e_s8(st):
                img, ti = st["img"], st["ti"]
                p0 = ti * TILE
                loc = st["local"][:, p0:p0 + TILE]
                nc.scalar.activation(loc, st["e2"], AF.Relu, bias=benc2_t)
                nc.vector.reduce_max(out=gparts[:, img, ti:ti + 1],
                                     in_=loc, axis=AX.X)

            E_STAGES = [e_s0, e_s1, e_s2, e_s3, e_s4, e_s5, e_s6, e_s7, e_s8]

            def h_s0(st):
                p0 = st["ti"] * TILE
                h0 = ps.tile([128, TILE], F32, tag="big")
                local_t = st["local"]
                for half in range(2):
                    sl = slice(p0 + half * 512, p0 + half * 512 + 512)
                    dsl = slice(half * 512, half * 512 + 512)
                    nc.tensor.matmul(h0[:, dsl], w0a_t, local_t[:, sl],
                                     start=True, stop=True)
                st["h0"] = h0

            def h_s1(st):
                y0 = wp.tile([128, TILE], BF16, tag="y0")
                nc.scalar.activation(y0, st["h0"], AF.Relu, bias=st["sh"]["b0h"])
                st["y0"] = y0

            def h_s2(st):
                h1 = ps.tile([64, TILE], F32, tag="big")
                y0 = st["y0"]
                for half in range(2):
                    sl = slice(half * 512, half * 512 + 512)
                    nc.tensor.matmul(h1[:, sl], wh1_t, y0[:, sl],
                                     start=True, stop=True)
                st["h1"] = h1

            def h_s3(st):
                y1 = wp.tile([64, TILE], BF16, tag="y1")
                nc.scalar.activation(y1, st["h1"], AF.Relu, bias=bh1_t)
                st["y1"] = y1

            def h_s4(st):
                wz = psb.tile([128, NCH], F32, tag="small")
                y1 = st["y1"]
                for j in range(NCH):
                    nc.tensor.matmul(wz[:, j:j + 1],
                                     y1[:, j * 128:(j + 1) * 128],
                                     w2col_t, start=True, stop=True)
                st["wz"] = wz

            def h_s5(st):
                img, ti = st["img"], st["ti"]
                wt = wp.tile([128, NCH], F32, tag="wt")
                nc.scalar.activation(wt, st["wz"], AF.Tanh, bias=tb2_t,
                                     scale=0.5)
                nc.vector.tensor_scalar(
                    w_all[:, img, ti * NCH:(ti + 1) * NCH], wt, 0.5, 0.5,
                    op0=OP.mult, op1=OP.add)

            H_STAGES = [h_s0, h_s1, h_s2, h_s3, h_s4, h_s5]

            def gram_img(img):
                # build q/qw on GpSimd (idle engine; SBUF-only work), then
                # accumulate M = Qw^T Q on TensorE in fp32
                pn = posn_sb[:, img]
                q = q_all[:, img]
                qw = qw_all[:, img]
                nc.gpsimd.tensor_copy(q[:, :, 0], pn[:, :, 0])
                nc.gpsimd.tensor_copy(q[:, :, 1], pn[:, :, 1])
                nc.gpsimd.memset(q[:, :, 2], 1.0)
                nc.gpsimd.tensor_copy(q[:, :, 3], pn[:, :, 2])
                nc.gpsimd.tensor_copy(q[:, :, 4], pn[:, :, 3])
                nc.gpsimd.tensor_tensor(out=q[:, :, 5], in0=pn[:, :, 2],
                                        in1=pn[:, :, 0], op=OP.mult)
                nc.gpsimd.tensor_tensor(out=q[:, :, 6], in0=pn[:, :, 2],
                                        in1=pn[:, :, 1], op=OP.mult)
                nc.gpsimd.tensor_tensor(out=q[:, :, 7], in0=pn[:, :, 3],
                                        in1=pn[:, :, 0], op=OP.mult)
                nc.gpsimd.tensor_tensor(out=q[:, :, 8], in0=pn[:, :, 3],
                                        in1=pn[:, :, 1], op=OP.mult)
                for k in range(9):
                    nc.gpsimd.tensor_tensor(out=qw[:, :, k], in0=q[:, :, k],
                                            in1=w_all[:, img], op=OP.mult)
                gm_ps = psb.tile([9, 9], F32, tag="small")
                for c in range(NC32):
                    nc.tensor.matmul(gm_ps, qw[:, c, :], q[:, c, :],
                                     start=(c == 0), stop=(c == NC32 - 1))
                gm_sb = wp.tile([9, 9], F32, tag="gm")
                nc.vector.tensor_copy(gm_sb, gm_ps)
                nc.sync.dma_start(out=out[img], in_=gm_sb)

            vp_all = pp.tile([128, NTC * NCH], F32)
            u_all = pp.tile([128, NTC * NCH], F32)
            q_all = pp.tile([128, BL, NC32, 9], F32)
            qw_all = pp.tile([128, BL, NC32, 9], F32)

            # windowed wavefront: units (stage lists) emitted round-robin
            # W at a time, so each in-order engine stream interleaves
            # independent work from neighbouring units
            def run_window(units, W=3):
                active = []
                idx = 0
                while idx < len(units) or active:
                    while len(active) < W and idx < len(units):
                        stages, st = units[idx]
                        active.append([stages, st, 0])
                        idx += 1
                    for u in list(active):
                        stages, st, k = u
                        stages[k](st)
                        u[2] += 1
                        if u[2] >= len(stages):
                            active.remove(u)

            def p1_s_load(st):
                st["f"] = p1_load(st["img"], st["ti"])

            def glob_s0(sh):
                img = sh["img"]
                glob_bf = wp.tile([128, 1], BF16, tag="glob")
                nc.vector.reduce_max(out=glob_bf, in_=gparts[:, img],
                                     axis=AX.X)
                gv_ps = psb.tile([128, NCH], F32, tag="small")
                nc.tensor.matmul(gv_ps[:, 0:1], w0b_t, glob_bf,
                                 start=True, stop=True)
                b0h = wp.tile([128, 1], F32, tag="b0h")
                nc.vector.tensor_scalar(b0h, gv_ps[:, 0:1], bh0_t, None,
                                        op0=OP.add)
                sh["b0h"] = b0h

            def newton_unit(img):
                return ([lambda st: newton_img(st["img"])], {"img": img})

            def newton_img(img):
                c0, c1 = img * NT * NCH, (img + 1) * NT * NCH
                s2f = s2_all.rearrange("p a b c -> p (a b c)")[:, c0:c1]
                vp = vp_all[:, c0:c1]
                yv = rstd_all[:, c0:c1]
                u_t = u_all[:, c0:c1]
                nc.vector.tensor_scalar(vp, s2f, 1.0 / 64.0, EPS,
                                        op0=OP.mult, op1=OP.add)
                nc.vector.tensor_scalar(yv.bitcast(I32), vp.bitcast(I32), 1,
                                        None, op0=OP.arith_shift_right)
                nc.vector.tensor_scalar(yv.bitcast(I32), yv.bitcast(I32),
                                        0xFFFFFFFF, None, op0=OP.bitwise_xor)
                nc.vector.tensor_scalar(yv.bitcast(I32), yv.bitcast(I32),
                                        MAGIC + 1, None, op0=OP.add)
                for _ in range(3):
                    nc.vector.tensor_mul(u_t, yv, yv)
                    nc.vector.tensor_mul(u_t, u_t, vp)
                    nc.vector.tensor_scalar(u_t, u_t, -0.5, 1.5,
                                            op0=OP.mult, op1=OP.add)
                    nc.vector.tensor_mul(yv, yv, u_t)

            def p1_units(img):
                us = []
                for ti in range(NT):
                    st = {"img": img, "ti": ti, "t": img * NT + ti}
                    us.append(([p1_s_load] + P1_STAGES, st))
                return us

            def enc_units(img, local_t):
                return [(E_STAGES, {"img": img, "ti": ti,
                                    "t": img * NT + ti, "local": local_t})
                        for ti in range(NT)]

            def head_units(img, local_t, sh):
                us = []
                for ti in range(NT):
                    st = {"img": img, "ti": ti, "local": local_t,
                          "sh": sh}
                    us.append((H_STAGES, st))
                return us

            def gram_unit(img):
                return ([lambda st: gram_img(st["img"])], {"img": img})

            def interleave(a, b):
                # spread b's units evenly between a's
                outu = []
                la, lb = len(a), len(b)
                bi = 0
                for i, u in enumerate(a):
                    outu.append(u)
                    want = (i + 1) * lb // la
                    while bi < want:
                        outu.append(b[bi])
                        bi += 1
                outu.extend(b[bi:])
                return outu

            units = []
            for img in range(BL):
                units += p1_units(img) + [newton_unit(img)]
            locals_ = {}
            for img in range(BL):
                local_t = fp.tile([128, N], BF16, tag="local")
                locals_[img] = local_t
                sh = {"img": img}
                units += (enc_units(img, locals_[img])
                          + [([glob_s0], sh)]
                          + head_units(img, locals_[img], sh)
                          + [gram_unit(img)])
            run_window(units, W=2)

    nc.compile()
    return nc


_CACHE = {}


def _get_nc():
    if "nc" not in _CACHE:
        _CACHE["nc"] = build()
    return _CACHE["nc"]


def _hartley(pts):
    """float32 numpy mirror of reference.hartley_normalize.
    Returns pts_norm [B,N,2], s [B], cx [B], cy [B]."""
    pts = pts.astype(np.float32)
    centroid = pts.mean(axis=1, keepdims=True)
    pc = pts - centroid
    dist = np.sqrt(np.clip((pc ** 2).sum(-1), 0.0, None))
    mean_dist = dist.mean(axis=1, keepdims=True)
    scale = np.float32(np.sqrt(2.0)) / np.clip(mean_dist, 0.001, None)
    scale = np.where(mean_dist < 0.001, np.ones_like(scale), scale)
    pts_norm = pc * scale[..., None]
    return (pts_norm.astype(np.float32), scale[:, 0].astype(np.float32),
            centroid[:, 0, 0].astype(np.float32),
            centroid[:, 0, 1].astype(np.float32))


def kernel(pos_A, pos_B, feat_A, feat_B,
           fc_w1, fc_b1, fc_ln_g, fc_ln_b, fc_w2, fc_b2,
           enc_w0, enc_g0, enc_b0, enc_w1, enc_g1, enc_b1,
           enc_w2, enc_g2, enc_b2,
           head_w0, head_g0, head_b0, head_w1, head_g1, head_b1,
           head_w2, head_b2):
    f32 = np.float32
    pos_A = np.asarray(pos_A, f32)
    pos_B = np.asarray(pos_B, f32)

    # ---- host prep: weights ----
    bnsc = f32(1.0 / np.sqrt(1.0 + EPS))
    w1c = (fc_w1 - fc_w1.mean(axis=0, keepdims=True)).astype(f32)
    b1c = (fc_b1 - fc_b1.mean()).astype(f32)
    s0 = (enc_g0 * bnsc).astype(f32)
    s1 = (enc_g1 * bnsc).astype(f32)
    s2 = (enc_g2 * bnsc).astype(f32)
    sh0 = (head_g0 * bnsc).astype(f32)
    sh1 = (head_g1 * bnsc).astype(f32)
    enc_w0s = (enc_w0 * s0[:, None]).astype(f32)
    enc_w1s = (enc_w1 * s1[:, None]).astype(f32)
    enc_w2s = (enc_w2 * s2[:, None]).astype(f32)
    head_w0s = (head_w0 * sh0[:, None]).astype(f32)
    head_w1s = (head_w1 * sh1[:, None]).astype(f32)
    wfold = (enc_w0s[:, 4:36] @ fc_w2).astype(f32)         # [128, 64]
    benc0 = (enc_b0 + enc_w0s[:, 4:36] @ fc_b2).astype(f32)
    wenc0 = np.concatenate([wfold.T, enc_w0s[:, 0:4].T], axis=0)  # [68,128]

    params = {
        "w1dT": w1c[:, 0:128].T.astype(BF),
        "w1mT": w1c[:, 128:256].T.astype(BF),
        "b1c": b1c.reshape(64, 1),
        "g_col": fc_ln_g.astype(f32).reshape(64, 1),
        "b_ln": fc_ln_b.astype(f32).reshape(64, 1),
        "wenc0": wenc0.astype(BF),
        "benc0": benc0.reshape(128, 1),
        "wenc1": enc_w1s.T.astype(BF),
        "benc1": enc_b1.astype(f32).reshape(128, 1),
        "wenc2": enc_w2s.T.astype(BF),
        "benc2": enc_b2.astype(f32).reshape(128, 1),
        "w0a": head_w0s[:, 0:128].T.astype(BF),
        "w0b": head_w0s[:, 128:256].T.astype(BF),
        "bh0": head_b0.astype(f32).reshape(128, 1),
        "wh1": head_w1s.T.astype(BF),
        "bh1": head_b1.astype(f32).reshape(64, 1),
        "w2col": head_w2.reshape(64, 1).astype(BF),
        "tb2": np.full((128, 1), 0.5 * float(head_b2[0]), f32),
    }

    # ---- host prep: positions ----
    srcn, sA, cxA, cyA = _hartley(pos_A)
    dstn, sB, cxB, cyB = _hartley(pos_B)
    pn = np.concatenate([srcn, dstn], axis=-1)             # [B, N, 4]
    pn = pn.reshape(B, NC32, 128, 4).transpose(2, 0, 1, 3)  # [128,B,32,4]
    pn = np.ascontiguousarray(
        pn.reshape(128, NCORES, BL, NC32, 4).transpose(1, 0, 2, 3, 4))
    posT = np.concatenate([pos_A, pos_B], axis=-1).transpose(0, 2, 1)  # [B,4,N]
    posT = np.ascontiguousarray(posT).astype(BF)
    fA = np.ascontiguousarray(
        np.asarray(feat_A).astype(BF).transpose(0, 2, 1))
    fB = np.ascontiguousarray(
        np.asarray(feat_B).astype(BF).transpose(0, 2, 1))

    in_maps = []
    for i in range(NCORES):
        sl = slice(i * BL, (i + 1) * BL)
        m = {"featA": fA[sl], "featB": fB[sl], "posT": posT[sl],
             "posn": pn[i]}
        m.update(params)
        in_maps.append(m)

    nc = _get_nc()
    res = bass_utils.run_bass_kernel_spmd(nc, in_maps,
                                          core_ids=list(range(NCORES)))
    M = np.concatenate([res.results[i]["out"] for i in range(NCORES)],
                       axis=0).astype(f32)                 # [B, 9, 9]

    # ---- host post: assemble AtWA/AtWb, solve, compose ----
    u3 = [0, 1, 2]
    AtWA = np.zeros((B, 8, 8), f32)
    AtWA[:, 0:3, 0:3] = M[:, 0:3, 0:3]
    AtWA[:, 3:6, 3:6] = M[:, 0:3, 0:3]
    AtWA[:, 0:3, 6] = -M[:, u3, 5]
    AtWA[:, 0:3, 7] = -M[:, u3, 6]
    AtWA[:, 3:6, 6] = -M[:, u3, 7]
    AtWA[:, 3:6, 7] = -M[:, u3, 8]
    AtWA[:, 6, 0:3] = -M[:, u3, 5]
    AtWA[:, 7, 0:3] = -M[:, u3, 6]
    AtWA[:, 6, 3:6] = -M[:, u3, 7]
    AtWA[:, 7, 3:6] = -M[:, u3, 8]
    AtWA[:, 6, 6] = M[:, 5, 5] + M[:, 7, 7]
    AtWA[:, 6, 7] = M[:, 5, 6] + M[:, 7, 8]
    AtWA[:, 7, 6] = M[:, 6, 5] + M[:, 8, 7]
    AtWA[:, 7, 7] = M[:, 6, 6] + M[:, 8, 8]
    AtWb = np.zeros((B, 8), f32)
    AtWb[:, 0:3] = M[:, 3, 0:3]
    AtWb[:, 3:6] = M[:, 4, 0:3]
    AtWb[:, 6] = -(M[:, 3, 5] + M[:, 4, 7])
    AtWb[:, 7] = -(M[:, 3, 6] + M[:, 4, 8])
    AtWA += REG * np.eye(8, dtype=f32)[None]
    h_id = np.array([1, 0, 0, 0, 1, 0, 0, 0], f32)
    AtWb += REG * h_id[None]

    try:
        h8 = np.linalg.solve(AtWA, AtWb[..., None])[..., 0].astype(f32)
    except np.linalg.LinAlgError:
        h8 = np.zeros((B, 8), f32)
        for b in range(B):
            try:
                h8[b] = np.linalg.solve(AtWA[b], AtWb[b])
            except np.linalg.LinAlgError:
                h8[b] = np.nan
    finite = np.all(np.isfinite(h8), axis=-1, keepdims=True)
    h8 = np.where(finite, h8, h_id[None])
    H_norm = np.concatenate([h8, np.ones((B, 1), f32)], axis=-1)
    H_norm = H_norm.reshape(B, 3, 3)

    def tmat(s, cx, cy):
        T = np.zeros((B, 3, 3), f32)
        T[:, 0, 0] = s
        T[:, 1, 1] = s
        T[:, 0, 2] = -s * cx
        T[:, 1, 2] = -s * cy
        T[:, 2, 2] = 1.0
        return T

    T_src = tmat(sA, cxA, cyA)
    s_dst = np.clip(sB, 1e-6, None)
    T_dst_inv = np.zeros((B, 3, 3), f32)
    T_dst_inv[:, 0, 0] = 1.0 / s_dst
    T_dst_inv[:, 1, 1] = 1.0 / s_dst
    T_dst_inv[:, 0, 2] = (sB * cxB) / s_dst
    T_dst_inv[:, 1, 2] = (sB * cyB) / s_dst
    T_dst_inv[:, 2, 2] = 1.0

    H = (T_dst_inv @ (H_norm @ T_src)).astype(f32)
    H = H / np.clip(np.abs(H[:, 2:3, 2:3]), 1e-8, None)
    h33 = H[:, 2:3, 2:3]
    sgn = np.sign(h33)
    sgn = np.where(sgn == 0, np.ones_like(sgn), sgn)
    H = H / (np.clip(np.abs(h33), 1e-8, None) * sgn)
    H_finite = np.all(np.isfinite(H), axis=(-2, -1))
    a33 = np.abs(H[:, 2, 2])
    valid = H_finite & (a33 > 1e-4) & (a33 < 1e4)
    eye = np.eye(3, dtype=f32)
    H = np.where(valid[:, None, None], H, eye[None])
    return H.astype(f32)


# revision 26
# speedup vs baseline: 1.1896x; 1.1896x over previous
"""AgriMatcher Trainium2 kernel: point-matching network + weighted-DLT homography.

Strategy: pure data-parallel over batch B=64 across 8 NeuronCores (8 images
per core). The device computes the heavy network (feature-compression MLP,
PointNet encoder, weighting head) and accumulates, per image, the 9x9
weighted Gram matrix M = sum_n w_n q_n q_n^T with
q = [sx, sy, 1, dx, dy, dx*sx, dx*sy, dy*sx, dy*sy] over Hartley-normalized
points. The host assembles AtWA/AtWb from M, solves the 8x8 system, and
composes the final 3x3 homographies (O(B * 8^3) flops, negligible).

Numerics: matmuls in bf16 with fp32 PSUM accumulation; DLT Gram in fp32;
LayerNorm folded into centered fc1 weights (mean-free), rstd via
Newton-iterated fast inverse sqrt on the Vector engine; sigmoid computed as
0.5*tanh(0.5x)+0.5 so every ScalarE function used (gelu/tanh/relu/copy/abs)
lives in one activation table set (no table-switch stalls).
"""

import numpy as np
import ml_dtypes

import concourse.bass as bass
import concourse.mybir as mybir
import concourse.tile as tile
from concourse import bacc, bass_utils
from concourse.masks import make_identity

F32 = mybir.dt.float32
BF16 = mybir.dt.bfloat16
I32 = mybir.dt.int32
AF = mybir.ActivationFunctionType
OP = mybir.AluOpType
AX = mybir.AxisListType

B, N, C = 64, 4096, 128
HID, COMP = 128, 32
NCORES = 8
BL = B // NCORES          # images per core
TILE = 1024               # points per tile
NT = N // TILE            # tiles per image (4)
NCH = TILE // 128         # 128-pt chunks per tile (8)
NTC = BL * NT             # tiles per core (32)
NC32 = N // 128           # 128-pt chunks per image (32)
EPS = 1e-5
REG = 1e-4
MAGIC = 0x5F3759DF

BF = ml_dtypes.bfloat16


def build():
    nc = bacc.Bacc("TRN2", target_bir_lowering=False, debug=False,
                   num_devices=NCORES)

    featA = nc.dram_tensor("featA", [BL, C, N], BF16, kind="ExternalInput").ap()
    featB = nc.dram_tensor("featB", [BL, C, N], BF16, kind="ExternalInput").ap()
    posT = nc.dram_tensor("posT", [BL, 4, N], BF16, kind="ExternalInput").ap()
    posn = nc.dram_tensor("posn", [128, BL, NC32, 4], F32,
                          kind="ExternalInput").ap()
    # packed params (see host prep below for layouts)
    w1dT = nc.dram_tensor("w1dT", [128, 64], BF16, kind="ExternalInput").ap()
    w1mT = nc.dram_tensor("w1mT", [128, 64], BF16, kind="ExternalInput").ap()
    b1c = nc.dram_tensor("b1c", [64, 1], F32, kind="ExternalInput").ap()
    g_col = nc.dram_tensor("g_col", [64, 1], F32, kind="ExternalInput").ap()
    b_ln = nc.dram_tensor("b_ln", [64, 1], F32, kind="ExternalInput").ap()
    wenc0 = nc.dram_tensor("wenc0", [68, 128], BF16, kind="ExternalInput").ap()
    benc0 = nc.dram_tensor("benc0", [128, 1], F32, kind="ExternalInput").ap()
    wenc1 = nc.dram_tensor("wenc1", [128, 128], BF16, kind="ExternalInput").ap()
    benc1 = nc.dram_tensor("benc1", [128, 1], F32, kind="ExternalInput").ap()
    wenc2 = nc.dram_tensor("wenc2", [128, 128], BF16, kind="ExternalInput").ap()
    benc2 = nc.dram_tensor("benc2", [128, 1], F32, kind="ExternalInput").ap()
    w0a = nc.dram_tensor("w0a", [128, 128], BF16, kind="ExternalInput").ap()
    w0b = nc.dram_tensor("w0b", [128, 128], BF16, kind="ExternalInput").ap()
    bh0 = nc.dram_tensor("bh0", [128, 1], F32, kind="ExternalInput").ap()
    wh1 = nc.dram_tensor("wh1", [128, 64], BF16, kind="ExternalInput").ap()
    bh1 = nc.dram_tensor("bh1", [64, 1], F32, kind="ExternalInput").ap()
    w2col = nc.dram_tensor("w2col", [64, 1], BF16, kind="ExternalInput").ap()
    tb2 = nc.dram_tensor("tb2", [128, 1], F32, kind="ExternalInput").ap()

    out = nc.dram_tensor("out", [BL, 9, 9], F32, kind="ExternalOutput").ap()

    with tile.TileContext(nc) as tc:
        with (
            tc.tile_pool(name="const", bufs=1) as cp,
            tc.tile_pool(name="persist", bufs=1) as pp,
            tc.tile_pool(name="work", bufs=3) as wp,
            tc.tile_pool(name="feat", bufs=4) as fp,
            tc.tile_pool(name="ps", bufs=2, space="PSUM") as ps,
            tc.tile_pool(name="psb", bufs=2, space="PSUM") as psb,
        ):
            # ---- constants ----
            ident = cp.tile([128, 128], BF16)
            make_identity(nc, ident)

            def cload(ap_in, shape, dtype):
                t = cp.tile(shape, dtype, tag=ap_in.tensor.name)
                nc.sync.dma_start(out=t, in_=ap_in)
                return t

            w1dT_t = cload(w1dT, [128, 64], BF16)
            w1mT_t = cload(w1mT, [128, 64], BF16)
            b1c_t = cload(b1c, [64, 1], F32)
            g_col_t = cload(g_col, [64, 1], F32)
            b_ln_t = cload(b_ln, [64, 1], F32)
            wenc0_t = cload(wenc0, [68, 128], BF16)
            benc0_t = cload(benc0, [128, 1], F32)
            wenc1_t = cload(wenc1, [128, 128], BF16)
            benc1_t = cload(benc1, [128, 1], F32)
            wenc2_t = cload(wenc2, [128, 128], BF16)
            benc2_t = cload(benc2, [128, 1], F32)
            w0a_t = cload(w0a, [128, 128], BF16)
            w0b_t = cload(w0b, [128, 128], BF16)
            bh0_t = cload(bh0, [128, 1], F32)
            wh1_t = cload(wh1, [128, 64], BF16)
            bh1_t = cload(bh1, [64, 1], F32)
            w2col_t = cload(w2col, [64, 1], BF16)
            tb2_t = cload(tb2, [128, 1], F32)

            posn_sb = pp.tile([128, BL, NC32, 4], F32)
            nc.sync.dma_start(out=posn_sb, in_=posn)

            # ---- persistent state ----
            hc_all = pp.tile([128, NTC, NCH, 64], BF16)   # centered fc1 out (PM)
            s2_all = pp.tile([128, NTC, NCH, 1], F32)     # sum(hc^2) per point
            rstd_all = pp.tile([128, NTC * NCH], F32)
            w_all = pp.tile([128, BL, NC32], F32)         # per-point weights
            gparts = pp.tile([128, BL, NT], BF16)         # per-tile max partials

            def p1_load(img, ti):
                p0 = ti * TILE
                faT = fp.tile([128, TILE], BF16, tag="faT")
                fbT = fp.tile([128, TILE], BF16, tag="fbT")
                nc.sync.dma_start(out=faT, in_=featA[img, :, p0:p0 + TILE])
                nc.sync.dma_start(out=fbT, in_=featB[img, :, p0:p0 + TILE])
                return faT, fbT

            def p1_s0(st):
                faT, fbT = st["f"]
                d_t = wp.tile([128, TILE], BF16, tag="d")
                nc.vector.tensor_sub(d_t, faT, fbT)
                st["d"] = d_t

            def p1_s1(st):
                d_t = st["d"]
                nc.scalar.activation(d_t, d_t, AF.Abs)

            def p1_s2(st):
                faT, fbT = st["f"]
                m_t = wp.tile([128, TILE], BF16, tag="m")
                nc.vector.tensor_mul(m_t, faT, fbT)
                st["m"] = m_t

            def p1_s3(st):
                d_t, m_t = st["d"], st["m"]
                h_ps = ps.tile([64, TILE], F32, tag="big")
                for half in range(2):
                    sl = slice(half * 512, half * 512 + 512)
                    nc.tensor.matmul(h_ps[:, sl], w1dT_t, d_t[:, sl],
                                     start=True, stop=False)
                    nc.tensor.matmul(h_ps[:, sl], w1mT_t, m_t[:, sl],
                                     start=False, stop=True)
                st["h_ps"] = h_ps

            def p1_s4(st):
                h_sb = wp.tile([64, TILE], BF16, tag="h_sb")
                nc.scalar.activation(h_sb, st["h_ps"], AF.Identity,
                                     bias=b1c_t)
                st["h_sb"] = h_sb

            def p1_s5(st):
                hp_ps = psb.tile([128, NCH, 64], BF16, tag="tp")
                h_sb = st["h_sb"]
                for j in range(NCH):
                    nc.tensor.transpose(hp_ps[:, j, :],
                                        h_sb[:, j * 128:(j + 1) * 128],
                                        ident[:64, :64])
                st["hp_ps"] = hp_ps

            def p1_s6(st):
                t = st["t"]
                hcv = hc_all[:, t].rearrange("p a b -> p (a b)")
                nc.vector.tensor_copy(
                    hcv, st["hp_ps"].rearrange("p a b -> p (a b)"))
                sq = wp.tile([128, NCH * 64], BF16, tag="sq")
                nc.vector.tensor_mul(sq, hcv, hcv)
                nc.vector.reduce_sum(out=s2_all[:, t],
                                     in_=sq.rearrange("p (a b) -> p a b",
                                                      a=NCH), axis=AX.X)

            P1_STAGES = [p1_s0, p1_s1, p1_s2, p1_s3, p1_s4, p1_s5, p1_s6]

            def newton_all():
                # rstd = (s2/64 + eps)^-1/2 via fast-invsqrt + 3 Newton steps
                s2f = s2_all.rearrange("p a b c -> p (a b c)")
                vp = vp_all
                yv = rstd_all
                u_t = u_all
                nc.vector.tensor_scalar(vp, s2f, 1.0 / 64.0, EPS,
                                        op0=OP.mult, op1=OP.add)
                nc.vector.tensor_scalar(yv.bitcast(I32), vp.bitcast(I32), 1,
                                        None, op0=OP.arith_shift_right)
                nc.vector.tensor_scalar(yv.bitcast(I32), yv.bitcast(I32),
                                        0xFFFFFFFF, None, op0=OP.bitwise_xor)
                nc.vector.tensor_scalar(yv.bitcast(I32), yv.bitcast(I32),
                                        MAGIC + 1, None, op0=OP.add)
                for _ in range(3):
                    nc.vector.tensor_mul(u_t, yv, yv)
                    nc.vector.tensor_mul(u_t, u_t, vp)
                    nc.vector.tensor_scalar(u_t, u_t, -0.5, 1.5,
                                            op0=OP.mult, op1=OP.add)
                    nc.vector.tensor_mul(yv, yv, u_t)

            def e_s0(st):
                t = st["t"]
                diag_t = wp.tile([128, NCH, 128], BF16, tag="diag")
                ident_b = bass.AP(tensor=ident.tensor, offset=ident.offset,
                                  ap=[ident.ap[0], [0, NCH], ident.ap[1]])
                rstd_b = rstd_all[:, t * NCH:(t + 1) * NCH]
                rstd_bc = bass.AP(tensor=rstd_b.tensor, offset=rstd_b.offset,
                                  ap=[rstd_b.ap[0], rstd_b.ap[1], [0, 128]])
                nc.vector.tensor_tensor(out=diag_t, in0=ident_b, in1=rstd_bc,
                                        op=OP.mult)
                st["diag"] = diag_t

            def e_s1(st):
                t = st["t"]
                ycm_ps = psb.tile([64, TILE], BF16, tag="tp")
                diag_t = st["diag"]
                for j in range(NCH):
                    nc.tensor.transpose(ycm_ps[:, j * 128:(j + 1) * 128],
                                        hc_all[:, t, j, :], diag_t[:, j, :])
                st["ycm"] = ycm_ps

            def e_s2(st):
                img, ti = st["img"], st["ti"]
                p0 = ti * TILE
                hg = wp.tile([68, TILE], BF16, tag="hg")
                nc.sync.dma_start(out=hg[64:68, :],
                                  in_=posT[img, :, p0:p0 + TILE])
                nc.scalar.activation(hg[0:64, :], st["ycm"], AF.Gelu,
                                     bias=b_ln_t, scale=g_col_t)
                st["hg"] = hg

            def e_s3(st):
                e0 = ps.tile([128, TILE], F32, tag="big")
                hg = st["hg"]
                for half in range(2):
                    sl = slice(half * 512, half * 512 + 512)
                    nc.tensor.matmul(e0[:, sl], wenc0_t, hg[:, sl],
                                     start=True, stop=True)
                st["e0"] = e0

            def e_s4(st):
                x1 = wp.tile([128, TILE], BF16, tag="x1")
                nc.scalar.activation(x1, st["e0"], AF.Relu, bias=benc0_t)
                st["x1"] = x1

            def e_s5(st):
                e1 = ps.tile([128, TILE], F32, tag="big")
                x1 = st["x1"]
                for half in range(2):
                    sl = slice(half * 512, half * 512 + 512)
                    nc.tensor.matmul(e1[:, sl], wenc1_t, x1[:, sl],
                                     start=True, stop=True)
                st["e1"] = e1

            def e_s6(st):
                x2 = wp.tile([128, TILE], BF16, tag="x2")
                nc.vector.tensor_scalar(x2, st["e1"], benc1_t, 0.0,
                                        op0=OP.add, op1=OP.max)
                st["x2"] = x2

            def e_s7(st):
                e2 = ps.tile([128, TILE], F32, tag="big")
                x2 = st["x2"]
                for half in range(2):
                    sl = slice(half * 512, half * 512 + 512)
                    nc.tensor.matmul(e2[:, sl], wenc2_t, x2[:, sl],
                                     start=True, stop=True)
                st["e2"] = e2

            def e_s8(st):
                img, ti = st["img"], st["ti"]
                p0 = ti * TILE
                loc = st["local"][:, p0:p0 + TILE]
                nc.scalar.activation(loc, st["e2"], AF.Relu, bias=benc2_t)
                nc.vector.reduce_max(out=gparts[:, img, ti:ti + 1],
                                     in_=loc, axis=AX.X)

            E_STAGES = [e_s0, e_s1, e_s2, e_s3, e_s4, e_s5, e_s6, e_s7, e_s8]

            def h_s0(st):
                p0 = st["ti"] * TILE
                h0 = ps.tile([128, TILE], F32, tag="big")
                local_t = st["local"]
                for half in range(2):
                    sl = slice(p0 + half * 512, p0 + half * 512 + 512)
                    dsl = slice(half * 512, half * 512 + 512)
                    nc.tensor.matmul(h0[:, dsl], w0a_t, local_t[:, sl],
                                     start=True, stop=True)
                st["h0"] = h0

            def h_s1(st):
                y0 = wp.tile([128, TILE], BF16, tag="y0")
                nc.scalar.activation(y0, st["h0"], AF.Relu, bias=st["sh"]["b0h"])
                st["y0"] = y0

            def h_s2(st):
                h1 = ps.tile([64, TILE], F32, tag="big")
                y0 = st["y0"]
                for half in range(2):
                    sl = slice(half * 512, half * 512 + 512)
                    nc.tensor.matmul(h1[:, sl], wh1_t, y0[:, sl],
                                     start=True, stop=True)
                st["h1"] = h1

            def h_s3(st):
                y1 = wp.tile([64, TILE], BF16, tag="y1")
                nc.scalar.activation(y1, st["h1"], AF.Relu, bias=bh1_t)
                st["y1"] = y1

            def h_s4(st):
                wz = psb.tile([128, NCH], F32, tag="small")
                y1 = st["y1"]
                for j in range(NCH):
                    nc.tensor.matmul(wz[:, j:j + 1],
                                     y1[:, j * 128:(j + 1) * 128],
                                     w2col_t, start=True, stop=True)
                st["wz"] = wz

            def h_s5(st):
                img, ti = st["img"], st["ti"]
                wt = wp.tile([128, NCH], F32, tag="wt")
                nc.scalar.activation(wt, st["wz"], AF.Tanh, bias=tb2_t,
                                     scale=0.5)
                nc.vector.tensor_scalar(
                    w_all[:, img, ti * NCH:(ti + 1) * NCH], wt, 0.5, 0.5,
                    op0=OP.mult, op1=OP.add)

            H_STAGES = [h_s0, h_s1, h_s2, h_s3, h_s4, h_s5]

            def gram_img(img):
                # build q/qw on GpSimd (idle engine; SBUF-only work), then
                # accumulate M = Qw^T Q on TensorE in fp32
                pn = posn_sb[:, img]
                q = q_all[:, img]
                qw = qw_all[:, img]
                nc.gpsimd.tensor_copy(q[:, :, 0], pn[:, :, 0])
                nc.gpsimd.tensor_copy(q[:, :, 1], pn[:, :, 1])
                nc.gpsimd.memset(q[:, :, 2], 1.0)
                nc.gpsimd.tensor_copy(q[:, :, 3], pn[:, :, 2])
                nc.gpsimd.tensor_copy(q[:, :, 4], pn[:, :, 3])
                nc.gpsimd.tensor_tensor(out=q[:, :, 5], in0=pn[:, :, 2],
                                        in1=pn[:, :, 0], op=OP.mult)
                nc.gpsimd.tensor_tensor(out=q[:, :, 6], in0=pn[:, :, 2],
                                        in1=pn[:, :, 1], op=OP.mult)
                nc.gpsimd.tensor_tensor(out=q[:, :, 7], in0=pn[:, :, 3],
                                        in1=pn[:, :, 0], op=OP.mult)
                nc.gpsimd.tensor_tensor(out=q[:, :, 8], in0=pn[:, :, 3],
                                        in1=pn[:, :, 1], op=OP.mult)
                for k in range(9):
                    nc.gpsimd.tensor_tensor(out=qw[:, :, k], in0=q[:, :, k],
                                            in1=w_all[:, img], op=OP.mult)
                gm_ps = psb.tile([9, 9], F32, tag="small")
                for c in range(NC32):
                    nc.tensor.matmul(gm_ps, qw[:, c, :], q[:, c, :],
                                     start=(c == 0), stop=(c == NC32 - 1))
                gm_sb = wp.tile([9, 9], F32, tag="gm")
                nc.vector.tensor_copy(gm_sb, gm_ps)
                nc.sync.dma_start(out=out[img], in_=gm_sb)

            vp_all = pp.tile([128, NTC * NCH], F32)
            u_all = pp.tile([128, NTC * NCH], F32)
            q_all = pp.tile([128, BL, NC32, 9], F32)
            qw_all = pp.tile([128, BL, NC32, 9], F32)

            # windowed wavefront: units (stage lists) emitted round-robin
            # W at a time, so each in-order engine stream interleaves
            # independent work from neighbouring units
            def run_window(units, W=3):
                active = []
                idx = 0
                while idx < len(units) or active:
                    while len(active) < W and idx < len(units):
                        stages, st = units[idx]
                        active.append([stages, st, 0])
                        idx += 1
                    for u in list(active):
                        stages, st, k = u
                        stages[k](st)
                        u[2] += 1
                        if u[2] >= len(stages):
                            active.remove(u)

            def p1_s_load(st):
                st["f"] = p1_load(st["img"], st["ti"])

            def glob_s0(sh):
                img = sh["img"]
                glob_bf = wp.tile([128, 1], BF16, tag="glob")
                nc.vector.reduce_max(out=glob_bf, in_=gparts[:, img],
                                     axis=AX.X)
                gv_ps = psb.tile([128, NCH], F32, tag="small")
                nc.tensor.matmul(gv_ps[:, 0:1], w0b_t, glob_bf,
                                 start=True, stop=True)
                b0h = wp.tile([128, 1], F32, tag="b0h")
                nc.vector.tensor_scalar(b0h, gv_ps[:, 0:1], bh0_t, None,
                                        op0=OP.add)
                sh["b0h"] = b0h

            def newton_unit(img):
                return ([lambda st: newton_img(st["img"])], {"img": img})

            def newton_img(img):
                c0, c1 = img * NT * NCH, (img + 1) * NT * NCH
                s2f = s2_all.rearrange("p a b c -> p (a b c)")[:, c0:c1]
                vp = vp_all[:, c0:c1]
                yv = rstd_all[:, c0:c1]
                u_t = u_all[:, c0:c1]
                nc.vector.tensor_scalar(vp, s2f, 1.0 / 64.0, EPS,
                                        op0=OP.mult, op1=OP.add)
                nc.vector.tensor_scalar(yv.bitcast(I32), vp.bitcast(I32), 1,
                                        None, op0=OP.arith_shift_right)
                nc.vector.tensor_scalar(yv.bitcast(I32), yv.bitcast(I32),
                                        0xFFFFFFFF, None, op0=OP.bitwise_xor)
                nc.vector.tensor_scalar(yv.bitcast(I32), yv.bitcast(I32),
                                        MAGIC + 1, None, op0=OP.add)
                for _ in range(3):
                    nc.vector.tensor_mul(u_t, yv, yv)
                    nc.vector.tensor_mul(u_t, u_t, vp)
                    nc.vector.tensor_scalar(u_t, u_t, -0.5, 1.5,
                                            op0=OP.mult, op1=OP.add)
                    nc.vector.tensor_mul(yv, yv, u_t)

            def p1_units(img):
                us = []
                for ti in range(NT):
                    st = {"img": img, "ti": ti, "t": img * NT + ti}
                    us.append(([p1_s_load] + P1_STAGES, st))
                return us

            def enc_units(img, local_t):
                return [(E_STAGES, {"img": img, "ti": ti,
                                    "t": img * NT + ti, "local": local_t})
                        for ti in range(NT)]

            def head_units(img, local_t, sh):
                us = []
                for ti in range(NT):
                    st = {"img": img, "ti": ti, "local": local_t,
                          "sh": sh}
                    us.append((H_STAGES, st))
                return us

            def gram_unit(img):
                return ([lambda st: gram_img(st["img"])], {"img": img})

            def interleave(a, b):
                # spread b's units evenly between a's
                outu = []
                la, lb = len(a), len(b)
                bi = 0
                for i, u in enumerate(a):
                    outu.append(u)
                    want = (i + 1) * lb // la
                    while bi < want:
                        outu.append(b[bi])
                        bi += 1
                outu.extend(b[bi:])
                return outu

            # phase 1: paired with deep DMA prefetch
            p1_sts = [{"img": img, "ti": ti, "t": img * NT + ti}
                      for img in range(BL) for ti in range(NT)]
            for st in p1_sts[:4]:
                st["f"] = p1_load(st["img"], st["ti"])
            for i in range(0, len(p1_sts), 2):
                pair = p1_sts[i:i + 2]
                for st in p1_sts[i + 4:i + 6]:
                    st["f"] = p1_load(st["img"], st["ti"])
                for stg in P1_STAGES:
                    for st in pair:
                        stg(st)
            for img in range(BL):
                newton_img(img)
            locals_ = {}
            units = []
            for img in range(BL):
                local_t = fp.tile([128, N], BF16, tag="local")
                locals_[img] = local_t
                sh = {"img": img}
                units += (enc_units(img, locals_[img])
                          + [([glob_s0], sh)]
                          + head_units(img, locals_[img], sh)
                          + [gram_unit(img)])
            run_window(units, W=2)

    nc.compile()
    return nc


_CACHE = {}


def _get_nc():
    if "nc" not in _CACHE:
        _CACHE["nc"] = build()
    return _CACHE["nc"]


def _hartley(pts):
    """float32 numpy mirror of reference.hartley_normalize.
    Returns pts_norm [B,N,2], s [B], cx [B], cy [B]."""
    pts = pts.astype(np.float32)
    centroid = pts.mean(axis=1, keepdims=True)
    pc = pts - centroid
    dist = np.sqrt(np.clip((pc ** 2).sum(-1), 0.0, None))
    mean_dist = dist.mean(axis=1, keepdims=True)
    scale = np.float32(np.sqrt(2.0)) / np.clip(mean_dist, 0.001, None)
    scale = np.where(mean_dist < 0.001, np.ones_like(scale), scale)
    pts_norm = pc * scale[..., None]
    return (pts_norm.astype(np.float32), scale[:, 0].astype(np.float32),
            centroid[:, 0, 0].astype(np.float32),
            centroid[:, 0, 1].astype(np.float32))


def kernel(pos_A, pos_B, feat_A, feat_B,
           fc_w1, fc_b1, fc_ln_g, fc_ln_b, fc_w2, fc_b2,
           enc_w0, enc_g0, enc_b0, enc_w1, enc_g1, enc_b1,
           enc_w2, enc_g2, enc_b2,
           head_w0, head_g0, head_b0, head_w1, head_g1, head_b1,
           head_w2, head_b2):
    f32 = np.float32
    pos_A = np.asarray(pos_A, f32)
    pos_B = np.asarray(pos_B, f32)

    # ---- host prep: weights ----
    bnsc = f32(1.0 / np.sqrt(1.0 + EPS))
    w1c = (fc_w1 - fc_w1.mean(axis=0, keepdims=True)).astype(f32)
    b1c = (fc_b1 - fc_b1.mean()).astype(f32)
    s0 = (enc_g0 * bnsc).astype(f32)
    s1 = (enc_g1 * bnsc).astype(f32)
    s2 = (enc_g2 * bnsc).astype(f32)
    sh0 = (head_g0 * bnsc).astype(f32)
    sh1 = (head_g1 * bnsc).astype(f32)
    enc_w0s = (enc_w0 * s0[:, None]).astype(f32)
    enc_w1s = (enc_w1 * s1[:, None]).astype(f32)
    enc_w2s = (enc_w2 * s2[:, None]).astype(f32)
    head_w0s = (head_w0 * sh0[:, None]).astype(f32)
    head_w1s = (head_w1 * sh1[:, None]).astype(f32)
    wfold = (enc_w0s[:, 4:36] @ fc_w2).astype(f32)         # [128, 64]
    benc0 = (enc_b0 + enc_w0s[:, 4:36] @ fc_b2).astype(f32)
    wenc0 = np.concatenate([wfold.T, enc_w0s[:, 0:4].T], axis=0)  # [68,128]

    params = {
        "w1dT": w1c[:, 0:128].T.astype(BF),
        "w1mT": w1c[:, 128:256].T.astype(BF),
        "b1c": b1c.reshape(64, 1),
        "g_col": fc_ln_g.astype(f32).reshape(64, 1),
        "b_ln": fc_ln_b.astype(f32).reshape(64, 1),
        "wenc0": wenc0.astype(BF),
        "benc0": benc0.reshape(128, 1),
        "wenc1": enc_w1s.T.astype(BF),
        "benc1": enc_b1.astype(f32).reshape(128, 1),
        "wenc2": enc_w2s.T.astype(BF),
        "benc2": enc_b2.astype(f32).reshape(128, 1),
        "w0a": head_w0s[:, 0:128].T.astype(BF),
        "w0b": head_w0s[:, 128:256].T.astype(BF),
        "bh0": head_b0.astype(f32).reshape(128, 1),
        "wh1": head_w1s.T.astype(BF),
        "bh1": head_b1.astype(f32).reshape(64, 1),
        "w2col": head_w2.reshape(64, 1).astype(BF),
        "tb2": np.full((128, 1), 0.5 * float(head_b2[0]), f32),
    }

    # ---- host prep: positions ----
    srcn, sA, cxA, cyA = _hartley(pos_A)
    dstn, sB, cxB, cyB = _hartley(pos_B)
    pn = np.concatenate([srcn, dstn], axis=-1)             # [B, N, 4]
    pn = pn.reshape(B, NC32, 128, 4).transpose(2, 0, 1, 3)  # [128,B,32,4]
    pn = np.ascontiguousarray(
        pn.reshape(128, NCORES, BL, NC32, 4).transpose(1, 0, 2, 3, 4))
    posT = np.concatenate([pos_A, pos_B], axis=-1).transpose(0, 2, 1)  # [B,4,N]
    posT = np.ascontiguousarray(posT).astype(BF)
    fA = np.ascontiguousarray(
        np.asarray(feat_A).astype(BF).transpose(0, 2, 1))
    fB = np.ascontiguousarray(
        np.asarray(feat_B).astype(BF).transpose(0, 2, 1))

    in_maps = []
    for i in range(NCORES):
        sl = slice(i * BL, (i + 1) * BL)
        m = {"featA": fA[sl], "featB": fB[sl], "posT": posT[sl],
             "posn": pn[i]}
        m.update(params)
        in_maps.append(m)

    nc = _get_nc()
    res = bass_utils.run_bass_kernel_spmd(nc, in_maps,
                                          core_ids=list(range(NCORES)))
    M = np.concatenate([res.results[i]["out"] for i in range(NCORES)],
                       axis=0).astype(f32)                 # [B, 9, 9]

    # ---- host post: assemble AtWA/AtWb, solve, compose ----
    u3 = [0, 1, 2]
    AtWA = np.zeros((B, 8, 8), f32)
    AtWA[:, 0:3, 0:3] = M[:, 0:3, 0:3]
    AtWA[:, 3:6, 3:6] = M[:, 0:3, 0:3]
    AtWA[:, 0:3, 6] = -M[:, u3, 5]
    AtWA[:, 0:3, 7] = -M[:, u3, 6]
    AtWA[:, 3:6, 6] = -M[:, u3, 7]
    AtWA[:, 3:6, 7] = -M[:, u3, 8]
    AtWA[:, 6, 0:3] = -M[:, u3, 5]
    AtWA[:, 7, 0:3] = -M[:, u3, 6]
    AtWA[:, 6, 3:6] = -M[:, u3, 7]
    AtWA[:, 7, 3:6] = -M[:, u3, 8]
    AtWA[:, 6, 6] = M[:, 5, 5] + M[:, 7, 7]
    AtWA[:, 6, 7] = M[:, 5, 6] + M[:, 7, 8]
    AtWA[:, 7, 6] = M[:, 6, 5] + M[:, 8, 7]
    AtWA[:, 7, 7] = M[:, 6, 6] + M[:, 8, 8]
    AtWb = np.zeros((B, 8), f32)
    AtWb[:, 0:3] = M[:, 3, 0:3]
    AtWb[:, 3:6] = M[:, 4, 0:3]
    AtWb[:, 6] = -(M[:, 3, 5] + M[:, 4, 7])
    AtWb[:, 7] = -(M[:, 3, 6] + M[:, 4, 8])
    AtWA += REG * np.eye(8, dtype=f32)[None]
    h_id = np.array([1, 0, 0, 0, 1, 0, 0, 0], f32)
    AtWb += REG * h_id[None]

    try:
        h8 = np.linalg.solve(AtWA, AtWb[..., None])[..., 0].astype(f32)
    except np.linalg.LinAlgError:
        h8 = np.zeros((B, 8), f32)
        for b in range(B):
            try:
                h8[b] = np.linalg.solve(AtWA[b], AtWb[b])
            except np.linalg.LinAlgError:
                h8[b] = np.nan
    finite = np.all(np.isfinite(h8), axis=-1, keepdims=True)
    h8 = np.where(finite, h8, h_id[None])
    H_norm = np.concatenate([h8, np.ones((B, 1), f32)], axis=-1)
    H_norm = H_norm.reshape(B, 3, 3)

    def tmat(s, cx, cy):
        T = np.zeros((B, 3, 3), f32)
        T[:, 0, 0] = s
        T[:, 1, 1] = s
        T[:, 0, 2] = -s * cx
        T[:, 1, 2] = -s * cy
        T[:, 2, 2] = 1.0
        return T

    T_src = tmat(sA, cxA, cyA)
    s_dst = np.clip(sB, 1e-6, None)
    T_dst_inv = np.zeros((B, 3, 3), f32)
    T_dst_inv[:, 0, 0] = 1.0 / s_dst
    T_dst_inv[:, 1, 1] = 1.0 / s_dst
    T_dst_inv[:, 0, 2] = (sB * cxB) / s_dst
    T_dst_inv[:, 1, 2] = (sB * cyB) / s_dst
    T_dst_inv[:, 2, 2] = 1.0

    H = (T_dst_inv @ (H_norm @ T_src)).astype(f32)
    H = H / np.clip(np.abs(H[:, 2:3, 2:3]), 1e-8, None)
    h33 = H[:, 2:3, 2:3]
    sgn = np.sign(h33)
    sgn = np.where(sgn == 0, np.ones_like(sgn), sgn)
    H = H / (np.clip(np.abs(h33), 1e-8, None) * sgn)
    H_finite = np.all(np.isfinite(H), axis=(-2, -1))
    a33 = np.abs(H[:, 2, 2])
    valid = H_finite & (a33 > 1e-4) & (a33 < 1e4)
    eye = np.eye(3, dtype=f32)
    H = np.where(valid[:, None, None], H, eye[None])
    return H.astype(f32)
